# revision 1
# baseline (speedup 1.0000x reference)
"""GraphSAGE mean-aggregation layer on 8 Trainium2 NeuronCores (raw Bass).

Math: out = D^{-1} A (x @ W + b)  ==  (D^{-1} A x) @ W + mask (outer) b
where A is the (row=dest, col=src) adjacency from edge_index, D = row degrees,
mask[d] = 1 if deg[d] > 0 else 0 (zero-degree rows are exactly 0 in the ref).

Strategy (one SPMD program on 8 cores, dest nodes sharded):
  - Host: sort edges by dest, bucket into 128-dest windows (wpc per core), pad
    each window to T tiles of 128 edges. Per-edge weight 1/deg[dest] is folded
    into the selection matrix so PSUM accumulation yields D^{-1}Ax directly.
  - Device, per window: one indirect-DMA gather of T*128 source rows (one row
    per partition per tile), then per 128-edge tile a DVE-built weighted
    one-hot S (S[e,j] = (dst_local[e]==j)*w[e]) and a PE matmul S^T @ G
    accumulating into PSUM [128 dests, 256]; transpose + W matmul + masked
    bias (K=1 outer product), DMA 128 output rows out.
  - Raw bass engine programs with explicit semaphores: this toolchain allows
    only ONE sync wait per instruction, so all waits are standalone wait_ge.
"""

import numpy as np

import concourse.bass as bass
import concourse.mybir as mybir
from concourse.bass_utils import run_bass_kernel_spmd

P = 128
F = 256

N_NODES = 100000
N_CORES = 8
NPC = N_NODES // N_CORES  # dest rows per core


def build_nc(n_nodes, npc, n_tiles, x_dtype=mybir.dt.float32, repeat=1):
    """One SPMD Bass program; n_tiles = edge tiles per 128-dest window."""
    wpc = (npc + P - 1) // P
    T = n_tiles
    f = F
    kf = f // P  # 2 feature chunks of 128
    NG = 2  # gather buffers
    dt_f32 = mybir.dt.float32

    nc = bass.Bass()

    x_h = nc.declare_dram_parameter("x", [n_nodes, f], x_dtype, isOutput=False)
    idx_h = nc.declare_dram_parameter("srcidx", [P, wpc * T], mybir.dt.int32, isOutput=False)
    dw_h = nc.declare_dram_parameter("dw", [P, wpc * 2 * T], dt_f32, isOutput=False)
    msk_h = nc.declare_dram_parameter("maskw", [wpc, P], dt_f32, isOutput=False)
    w_h = nc.declare_dram_parameter("Wm", [f, f], dt_f32, isOutput=False)
    b_h = nc.declare_dram_parameter("bv", [1, f], dt_f32, isOutput=False)
    out_h = nc.declare_dram_parameter("out", [npc, f], dt_f32, isOutput=True)

    NS = T + 12  # S-tile ring: one window + pipeline margin

    from contextlib import ExitStack

    ctx = ExitStack()
    with ctx:
        sb = lambda name, shape, dt: ctx.enter_context(nc.sbuf_tensor(name, shape, dt))
        ps = lambda name, shape: ctx.enter_context(nc.psum_tensor(name, shape, dt_f32))
        sem = lambda name: ctx.enter_context(nc.semaphore(name))

        iota_f = sb("iota_f", [P, P], dt_f32)
        ident = sb("ident", [P, P], dt_f32)
        w0 = sb("w0", [P, f], dt_f32)
        w1 = sb("w1", [P, f], dt_f32)
        b_sb = sb("b_sb", [1, f], dt_f32)
        idx_all = sb("idx_all", [P, wpc * T], mybir.dt.int32)
        dw_all = sb("dw_all", [P, wpc * 2 * T], dt_f32)
        msk_t = sb("msk_t", [1, 2 * P], dt_f32)
        g_buf = sb("g_buf", [P, NG * T * f], x_dtype)
        s_buf = sb("s_buf", [P, NS * P], x_dtype)
        agg_sb = sb("agg_sb", [P, 2 * f], dt_f32)
        tp_sb = sb("tp_sb", [P, kf * P], dt_f32)
        out_sb = sb("out_sb", [P, 2 * f], dt_f32)
        agg_ps = [ps("agg_ps0", [P, f]), ps("agg_ps1", [P, f])]
        tp_ps = [ps("tp_ps0", [P, P]), ps("tp_ps1", [P, P])]
        out_ps = [ps("out_ps0", [P, f]), ps("out_ps1", [P, f])]
        SEM_META = sem("sem_meta")
        SEM_CONST = sem("sem_const")
        SEM_G = sem("sem_g")
        SEM_S = sem("sem_s")
        SEM_MM = sem("sem_mm")
        SEM_CP = sem("sem_cp")
        SEM_TP = sem("sem_tp")
        SEM_TPC = sem("sem_tpc")
        SEM_FIN = sem("sem_fin")
        SEM_OUT = sem("sem_out")
        SEM_OD = sem("sem_od")
        SEM_MSK = sem("sem_msk")

        w_sb = [w0, w1]

        with nc.Block() as block:

            @block.sync
            def _(sync):
                # startup loads (HWDGE)
                sync.dma_start(w0[:, :], w_h[0:P, :]).then_inc(SEM_META, 16)
                sync.dma_start(w1[:, :], w_h[P : 2 * P, :]).then_inc(SEM_META, 16)
                sync.dma_start(b_sb[:, :], b_h[:, :]).then_inc(SEM_META, 16)
                sync.dma_start(idx_all[:, :], idx_h[:, :]).then_inc(SEM_META, 16)
                sync.dma_start(dw_all[:, :], dw_h[:, :]).then_inc(SEM_META, 16)
                # per-window mask loads + output stores
                for W in range(repeat * wpc):
                    w = W % wpc
                    rows = min(P, npc - w * P)
                    ob = (W % 2) * f
                    mb = (W % 2) * P
                    if W >= 2:
                        sync.wait_ge(SEM_FIN, W - 1)  # msk_t slot free
                    sync.dma_start(
                        msk_t[:1, mb : mb + P], msk_h[w : w + 1, :]
                    ).then_inc(SEM_MSK, 16)
                    sync.wait_ge(SEM_OUT, W + 1)
                    sync.dma_start(
                        out_h[w * P : w * P + rows, :], out_sb[:rows, ob : ob + f]
                    ).then_inc(SEM_OD, 16)

            @block.gpsimd
            def _(gpsimd):
                # constants
                gpsimd.iota(
                    iota_f[:, :],
                    pattern=[[1, P]],
                    base=0,
                    channel_multiplier=0,
                    allow_small_or_imprecise_dtypes=True,
                )
                gpsimd.memset(ident[:, :], 0.0)
                gpsimd.affine_select(
                    out=ident[:, :],
                    in_=ident[:, :],
                    compare_op=mybir.AluOpType.not_equal,
                    fill=1.0,
                    base=0,
                    pattern=[[-1, P]],
                    channel_multiplier=1,
                ).then_inc(SEM_CONST, 1)
                # gathers
                gpsimd.wait_ge(SEM_META, 80)
                for W in range(repeat * wpc):
                    w = W % wpc
                    gb = (W % NG) * T * f
                    if W >= NG:
                        # g buffer free once PE finished window W-NG's matmuls
                        gpsimd.wait_ge(SEM_MM, (W - NG + 1) * T)
                    for t in range(T):
                        # HW indirect DMA honors ONE offset per partition:
                        # one call per 128-edge tile.
                        gpsimd.indirect_dma_start(
                            out=g_buf[:, gb + t * f : gb + (t + 1) * f],
                            out_offset=None,
                            in_=x_h[:, :],
                            in_offset=bass.IndirectOffsetOnAxis(
                                ap=idx_all[:, w * T + t : w * T + t + 1], axis=0
                            ),
                        ).then_inc(SEM_G, 16)

            @block.vector
            def _(vector):
                vector.wait_ge(SEM_CONST, 1)
                vector.wait_ge(SEM_META, 80)
                for W in range(repeat * wpc):
                    w = W % wpc
                    # build S tiles for window w
                    for t in range(T):
                        i = W * T + t
                        sb = (i % NS) * P
                        if i >= NS:
                            vector.wait_ge(SEM_MM, i - NS + 1)
                        vector.tensor_scalar(
                            out=s_buf[:, sb : sb + P],
                            in0=iota_f[:, :],
                            scalar1=dw_all[:, w * 2 * T + t : w * 2 * T + t + 1],
                            scalar2=dw_all[:, w * 2 * T + T + t : w * 2 * T + T + t + 1],
                            op0=mybir.AluOpType.is_equal,
                            op1=mybir.AluOpType.mult,
                        ).then_inc(SEM_S, 1)
                    # copy window aggregate out of PSUM
                    ab = (W % 2) * f
                    vector.wait_ge(SEM_MM, (W + 1) * T)
                    vector.tensor_copy(
                        agg_sb[:, ab : ab + f], agg_ps[W % 2][:, :]
                    ).then_inc(SEM_CP, 1)
                    # copy transposes out of PSUM
                    for k in range(kf):
                        vector.wait_ge(SEM_TP, kf * W + k + 1)
                        vector.tensor_copy(
                            tp_sb[:, k * P : (k + 1) * P], tp_ps[k][:, :]
                        ).then_inc(SEM_TPC, 1)
                    # copy final output out of PSUM
                    ob = (W % 2) * f
                    if W >= 2:
                        vector.wait_ge(SEM_OD, (W - 1) * 16)
                    vector.wait_ge(SEM_FIN, W + 1)
                    vector.tensor_copy(
                        out_sb[:, ob : ob + f], out_ps[W % 2][:, :]
                    ).then_inc(SEM_OUT, 1)

            @block.tensor
            def _(tensor):
                tensor.wait_ge(SEM_META, 80)
                tensor.wait_ge(SEM_CONST, 1)
                for W in range(repeat * wpc):
                    w = W % wpc
                    ab = (W % 2) * f
                    gb = (W % NG) * T * f
                    if W >= 2:
                        tensor.wait_ge(SEM_CP, W - 1)  # agg bank free
                    tensor.wait_ge(SEM_S, (W + 1) * T)  # all S of window ready
                    for t in range(T):
                        i = W * T + t
                        sb = (i % NS) * P
                        tensor.wait_ge(SEM_G, 16 * (i + 1))  # tile t gathered
                        tensor.matmul(
                            agg_ps[W % 2][:, :],
                            s_buf[:, sb : sb + P],
                            g_buf[:, gb + t * f : gb + (t + 1) * f],
                            start=(t == 0),
                            stop=(t == T - 1),
                        ).then_inc(SEM_MM, 1)
                    tensor.wait_ge(SEM_CP, W + 1)  # agg_sb ready
                    for k in range(kf):
                        if W >= 1:
                            tensor.wait_ge(SEM_TPC, kf * (W - 1) + k + 1)  # tp bank free
                        tensor.transpose(
                            tp_ps[k][:, :],
                            agg_sb[:, ab + k * P : ab + (k + 1) * P],
                            ident[:, :],
                        ).then_inc(SEM_TP, 1)
                    ob = (W % 2) * f
                    if W >= 2:
                        tensor.wait_ge(SEM_OUT, W - 1)  # out_ps bank free
                    for k in range(kf):
                        tensor.wait_ge(SEM_TPC, kf * W + k + 1)  # tp_sb ready
                        tensor.matmul(
                            out_ps[W % 2][:, :],
                            tp_sb[:, k * P : (k + 1) * P],
                            w_sb[k][:, :],
                            start=(k == 0),
                            stop=False,
                        )
                    tensor.wait_ge(SEM_MSK, 16 * (W + 1))
                    tensor.matmul(
                        out_ps[W % 2][:, :],
                        msk_t[:1, (W % 2) * P : (W % 2) * P + P],
                        b_sb[:1, :],
                        start=False,
                        stop=True,
                    ).then_inc(SEM_FIN, 1)

    return nc


def prepare_inputs(x, edge_index, W, b, n_cores=N_CORES):
    """Host-side: sort/bucket edges by destination into per-core padded windows."""
    n = x.shape[0]
    npc = n // n_cores
    wpc = (npc + P - 1) // P

    row = np.asarray(edge_index[0], dtype=np.int64)  # dest
    col = np.asarray(edge_index[1], dtype=np.int64)  # src

    deg = np.bincount(row, minlength=n).astype(np.float32)
    invdeg = np.zeros(n, dtype=np.float32)
    nz = deg > 0
    invdeg[nz] = 1.0 / deg[nz]

    order = np.argsort(row, kind="stable")
    row_s = row[order]
    col_s = col[order]

    core_of = row_s // npc
    local = row_s - core_of * npc
    win = local // P
    dstl = local % P
    gwin = core_of * wpc + win
    n_gw = n_cores * wpc

    counts = np.bincount(gwin, minlength=n_gw)
    n_tiles = max(1, int(np.ceil(counts.max() / P)))
    T = n_tiles

    first = np.searchsorted(gwin, np.arange(n_gw))
    pos = np.arange(len(gwin)) - first[gwin]
    t_of = pos // P
    p_of = pos % P

    srcidx = np.zeros((n_cores, wpc, P, T), dtype=np.int32)
    dstloc = np.full((n_cores, wpc, P, 2 * T), -1.0, dtype=np.float32)

    srcidx[core_of, win, p_of, t_of] = col_s.astype(np.int32)
    dstloc[core_of, win, p_of, t_of] = dstl.astype(np.float32)
    dstloc[core_of, win, p_of, T + t_of] = invdeg[row_s]

    maskw = np.zeros((n_cores, wpc * P), dtype=np.float32)
    maskw[:, :npc] = nz.astype(np.float32).reshape(n_cores, npc)
    maskw = maskw.reshape(n_cores, wpc, P)

    x_c = np.ascontiguousarray(x, dtype=mybir.dt.np(mybir.dt.float32))
    per_core = []
    for c in range(n_cores):
        per_core.append(
            {
                "x": x_c,
                "srcidx": np.ascontiguousarray(
                    srcidx[c].transpose(1, 0, 2).reshape(P, wpc * T)
                ),
                "dw": np.ascontiguousarray(
                    dstloc[c].transpose(1, 0, 2).reshape(P, wpc * 2 * T)
                ),
                "maskw": maskw[c],
                "Wm": np.ascontiguousarray(W, dtype=np.float32),
                "bv": np.ascontiguousarray(b, dtype=np.float32).reshape(1, -1),
            }
        )
    return per_core, n_tiles


def run(x, edge_index, W, b, n_cores=N_CORES, trace=False):
    n, f = x.shape
    npc = n // n_cores
    in_maps, n_tiles = prepare_inputs(x, edge_index, W, b, n_cores)
    nc = build_nc(n, npc, n_tiles)
    res = run_bass_kernel_spmd(nc, in_maps, list(range(n_cores)), trace=trace)
    out = np.concatenate([res.results[c]["out"] for c in range(n_cores)], axis=0)
    return out, res


def kernel(x, edge_index, W, b):
    out, _ = run(np.asarray(x), np.asarray(edge_index), np.asarray(W), np.asarray(b))
    return out.astype(np.float32)



# revision 12
# speedup vs baseline: 1.2120x; 1.2120x over previous
"""GraphSAGE mean-aggregation layer on 8 Trainium2 NeuronCores (raw Bass).

Math: out = D^{-1} A (x @ W + b)  ==  (D^{-1} A x) @ W + s (outer) b
where A is the (row=dest, col=src) adjacency from edge_index, D = row degrees,
s[d] = 1 if deg[d] > 0 else 0 (zero-degree rows are exactly 0 in the ref).

Strategy (one SPMD program on 8 cores, dest nodes sharded):
  - Host: sort edges by (dest window, src chunk). x is cast to fp16 and src
    rows are fetched with the gpsimd dma_gather custom instruction (int16
    indices => x is split into 4 chunks of 25000 rows). Each gather call
    covers one (window-pair, chunk) segment, capacity-padded with idx-0 /
    weight-0 edges so the SPMD program is identical on all cores.
  - Device, per 128-dest window: per 128-edge tile a DVE-built weighted
    one-hot S (S[e,j] = (dst_local[e]==j)*invdeg) in fp16, and PE matmuls
    G_half^T @ S accumulating the TRANSPOSED aggregate aggT [feat, dest]
    directly in PSUM (no separate transpose step). Scalar engine copies
    aggT to SBUF as fp16; PE then computes out = aggT^T @ W + s x b and the
    sync engine streams the result out.
"""

import numpy as np

import concourse.bass as bass
import concourse.mybir as mybir
from concourse import bacc
from concourse.bass_utils import run_bass_kernel_spmd

P = 128
F = 256

N_NODES = 100000
N_CORES = 8
NPC = N_NODES // N_CORES  # dest rows per core (12500)
WPC = (NPC + P - 1) // P  # dest windows per core (98)
ROWS_LAST = NPC - (WPC - 1) * P  # valid rows in last window (84)
NCH = 4
CH = N_NODES // NCH  # chunk rows (25000), int16-indexable
GW = 2  # windows per gather group
NGRP = WPC // GW  # gather groups per core (49)


def build_nc(t_c, have_mask):
    """One SPMD Bass program. t_c: tiles per chunk segment (len 4)."""
    t_c = list(t_c)
    T = sum(t_c)  # tiles per window
    tile_base = [0]
    for c in range(NCH):
        tile_base.append(tile_base[-1] + t_c[c])
    SLOTS_G = GW * T  # gather-buffer slots per group
    CPG = sum((2 * t_c[c] + 7) // 8 for c in range(NCH))  # gather calls per group
    NS = 3 * T  # S-tile ring slots
    IDXCOLS = NGRP * 16 * T
    DWCOLS = WPC * T * 2

    dt_f32 = mybir.dt.float32
    dt_f16 = mybir.dt.float16
    dt_i16 = mybir.dt.int16

    nc = bacc.Bacc()

    x_h = nc.declare_dram_parameter("x", [N_NODES, F], dt_f16, isOutput=False)
    idx_h = nc.declare_dram_parameter("srcidx", [P, IDXCOLS], dt_i16, isOutput=False)
    dw_h = nc.declare_dram_parameter("dw", [P, DWCOLS], dt_f32, isOutput=False)
    w_h = nc.declare_dram_parameter("Wm", [F, F], dt_f16, isOutput=False)
    b_h = nc.declare_dram_parameter("bv", [1, F], dt_f16, isOutput=False)
    if have_mask:
        msk_h = nc.declare_dram_parameter("maskw", [1, WPC * P], dt_f16, isOutput=False)
    out_h = nc.declare_dram_parameter("out", [NPC, F], dt_f32, isOutput=True)

    n_loads = 6 if have_mask else 5
    META_TGT = 16 * n_loads

    from contextlib import ExitStack

    ctx = ExitStack()
    with ctx:
        sb = lambda name, shape, dt: ctx.enter_context(nc.sbuf_tensor(name, shape, dt))
        ps = lambda name, shape: ctx.enter_context(nc.psum_tensor(name, shape, dt_f32))
        sem = lambda name: ctx.enter_context(nc.semaphore(name))

        iota16 = sb("iota16", [P, P], dt_f16)
        ones16 = sb("ones16", [1, P], dt_f16)
        w0 = sb("w0", [P, F], dt_f16)
        w1 = sb("w1", [P, F], dt_f16)
        b_sb = sb("b_sb", [1, F], dt_f16)
        if have_mask:
            msk_sb = sb("msk_sb", [1, WPC * P], dt_f16)
        idx_sb = sb("idx_sb", [P, IDXCOLS], dt_i16)
        dw_sb = sb("dw_sb", [P, DWCOLS], dt_f32)
        g_buf = sb("g_buf", [P, 2 * SLOTS_G * F], dt_f16)
        s_buf = sb("s_buf", [P, NS * P], dt_f16)
        aggT_sb = sb("aggT_sb", [P, 2 * 2 * P], dt_f16)  # [pair][half]
        out_sb = sb("out_sb", [P, 2 * F], dt_f32)
        aggT_ps = [
            [ps("aggT_ps00", [P, P]), ps("aggT_ps01", [P, P])],
            [ps("aggT_ps10", [P, P]), ps("aggT_ps11", [P, P])],
        ]
        out_ps = [ps("out_ps0", [P, F]), ps("out_ps1", [P, F])]

        SEM_META = sem("sem_meta")
        SEM_CONST = sem("sem_const")
        SEM_ONES = sem("sem_ones")
        SEM_G = [sem("sem_g0"), sem("sem_g1")]
        SEM_S = sem("sem_s")
        SEM_MM = sem("sem_mm")
        SEM_CP = sem("sem_cp")
        SEM_FIN = sem("sem_fin")
        SEM_OUT = sem("sem_out")
        SEM_OD = [sem("sem_od0"), sem("sem_od1")]

        w_sb = [w0, w1]

        def gslot(g, c, j, k):
            """Gather-buffer slot of window-pair-member j, chunk c, tile k."""
            return (g % 2) * SLOTS_G + 2 * tile_base[c] + j * t_c[c] + k

        with nc.Block() as block:

            @block.sync
            def _(sync):
                sync.dma_start(w0[:, :], w_h[0:P, :]).then_inc(SEM_META, 16)
                sync.dma_start(w1[:, :], w_h[P : 2 * P, :]).then_inc(SEM_META, 16)
                sync.dma_start(b_sb[:, :], b_h[:, :]).then_inc(SEM_META, 16)
                sync.dma_start(idx_sb[:, :], idx_h[:, :]).then_inc(SEM_META, 16)
                sync.dma_start(dw_sb[:, :], dw_h[:, :]).then_inc(SEM_META, 16)
                if have_mask:
                    sync.dma_start(msk_sb[:, :], msk_h[:, :]).then_inc(SEM_META, 16)
                for w in range(WPC):
                    rows = P if w < WPC - 1 else ROWS_LAST
                    ob = (w % 2) * F
                    sync.wait_ge(SEM_OUT, w + 1)
                    sync.dma_start(
                        out_h[w * P : w * P + rows, :], out_sb[:rows, ob : ob + F]
                    ).then_inc(SEM_OD[w % 2], 16)

            @block.gpsimd
            def _(gpsimd):
                gpsimd.iota(
                    iota16[:, :],
                    pattern=[[1, P]],
                    base=0,
                    channel_multiplier=0,
                    allow_small_or_imprecise_dtypes=True,
                ).then_inc(SEM_CONST, 1)
                gpsimd.wait_ge(SEM_META, META_TGT)
                ni_regs = {}
                for c in range(NCH):
                    for off in range(0, 2 * t_c[c], 8):
                        nt = min(8, 2 * t_c[c] - off)
                        if nt * P not in ni_regs:
                            ni_regs[nt * P] = gpsimd.to_reg(nt * P)
                for g in range(NGRP):
                    if g >= 2:
                        # group buffer g%2 free once windows 2g-4, 2g-3 consumed
                        gpsimd.wait_ge(SEM_MM, (2 * g - 2) * 2 * T)
                    for c in range(NCH):
                        base = gslot(g, c, 0, 0)
                        col0 = g * 16 * T + 16 * tile_base[c]
                        # dma_gather is limited to 1024 indices per call
                        for off in range(0, 2 * t_c[c], 8):
                            nt = min(8, 2 * t_c[c] - off)
                            ni = nt * P
                            out_ap = g_buf[
                                :, (base + off) * F : (base + off + nt) * F
                            ].rearrange("p (s e) -> p s e", e=F)
                            gpsimd.dma_gather(
                                out_ap,
                                x_h[c * CH : (c + 1) * CH, :],
                                idx_sb[:, col0 + off * 8 : col0 + off * 8 + nt * 8],
                                ni,
                                ni_regs[ni],
                                F,
                            ).then_inc(SEM_G[g % 2], 16)

            @block.vector
            def _(vector):
                vector.memset(ones16[:, :], 1.0).then_inc(SEM_ONES, 1)
                vector.wait_ge(SEM_META, META_TGT)
                vector.wait_ge(SEM_CONST, 1)
                for w in range(WPC):
                    if w >= 3:
                        vector.wait_ge(SEM_MM, (w - 2) * 2 * T)
                    for t in range(T):
                        i = w * T + t
                        s0 = (i % NS) * P
                        col = (w * T + t) * 2
                        vector.tensor_scalar(
                            out=s_buf[:, s0 : s0 + P],
                            in0=iota16[:, :],
                            scalar1=dw_sb[:, col : col + 1],
                            scalar2=dw_sb[:, col + 1 : col + 2],
                            op0=mybir.AluOpType.is_equal,
                            op1=mybir.AluOpType.mult,
                        ).then_inc(SEM_S, 1)

            @block.scalar
            def _(scalar):
                for w in range(WPC):
                    pair = w % 2
                    scalar.wait_ge(SEM_MM, (w + 1) * 2 * T)
                    for h in range(2):
                        a0 = (pair * 2 + h) * P
                        scalar.copy(
                            aggT_sb[:, a0 : a0 + P], aggT_ps[pair][h][:, :]
                        ).then_inc(SEM_CP, 1)
                    scalar.wait_ge(SEM_FIN, w + 1)
                    if w >= 2:
                        scalar.wait_ge(SEM_OD[w % 2], 16 * (w // 2))
                    scalar.copy(
                        out_sb[:, pair * F : (pair + 1) * F], out_ps[pair][:, :]
                    ).then_inc(SEM_OUT, 1)

            @block.tensor
            def _(tensor):
                def tail(w):
                    pair = w % 2
                    tensor.wait_ge(SEM_CP, 2 * w + 2)
                    if w >= 2:
                        tensor.wait_ge(SEM_OUT, w - 1)
                    for h in range(2):
                        a0 = (pair * 2 + h) * P
                        tensor.matmul(
                            out_ps[pair][:, :],
                            aggT_sb[:, a0 : a0 + P],
                            w_sb[h][:, :],
                            start=(h == 0),
                            stop=False,
                        )
                    stat = (
                        msk_sb[0:1, w * P : (w + 1) * P]
                        if have_mask
                        else ones16[0:1, :]
                    )
                    tensor.matmul(
                        out_ps[pair][:, :],
                        stat,
                        b_sb[0:1, :],
                        start=False,
                        stop=True,
                    ).then_inc(SEM_FIN, 1)

                tensor.wait_ge(SEM_META, META_TGT)
                tensor.wait_ge(SEM_CONST, 1)
                tensor.wait_ge(SEM_ONES, 1)
                for w in range(WPC):
                    g = w // 2
                    j = w % 2
                    pair = w % 2
                    if w >= 2:
                        tensor.wait_ge(SEM_CP, 2 * w - 2)  # aggT_ps[pair] free
                    tensor.wait_ge(SEM_S, (w + 1) * T)
                    tensor.wait_ge(SEM_G[g % 2], 16 * CPG * (g // 2 + 1))
                    for c in range(NCH):
                        for k in range(t_c[c]):
                            t = tile_base[c] + k
                            slot = gslot(g, c, j, k)
                            s0 = ((w * T + t) % NS) * P
                            st = t == 0
                            sp = t == T - 1
                            for h in range(2):
                                tensor.matmul(
                                    aggT_ps[pair][h][:, :],
                                    g_buf[:, slot * F + h * P : slot * F + (h + 1) * P],
                                    s_buf[:, s0 : s0 + P],
                                    start=st,
                                    stop=sp,
                                ).then_inc(SEM_MM, 1)
                    if w >= 1:
                        tail(w - 1)
                tail(WPC - 1)

    return nc


def prepare_inputs(x, edge_index, W, b):
    """Host-side: bucket edges by (core, window, chunk) into capacity-padded
    gather segments; build int16 gather-index tables and per-tile
    (dst_local, invdeg) scalar tables."""
    n = N_NODES
    row = np.asarray(edge_index[0], dtype=np.int64)  # dest
    col = np.asarray(edge_index[1], dtype=np.int64)  # src
    E = row.shape[0]

    deg = np.bincount(row, minlength=n)
    invdeg = np.where(deg > 0, 1.0 / np.maximum(deg, 1), 0.0).astype(np.float32)
    have_mask = bool((deg == 0).any())

    core = row // NPC
    local = row - core * NPC
    win = local >> 7
    dstl = (local & 127).astype(np.float32)
    chunk = col // CH
    cidx = (col - chunk * CH).astype(np.int16)

    bucket = (core * WPC + win) * NCH + chunk
    order = np.argsort(bucket, kind="stable")
    b_s = bucket[order]
    NB = N_CORES * WPC * NCH
    counts = np.bincount(b_s, minlength=NB)
    t_c = [
        int(np.ceil(counts.reshape(N_CORES, WPC, NCH)[:, :, c].max() / P))
        for c in range(NCH)
    ]
    T = sum(t_c)
    tile_base = np.concatenate([[0], np.cumsum(t_c)]).astype(np.int64)
    capE = np.array(t_c, dtype=np.int64) * P

    first = np.zeros(NB, dtype=np.int64)
    np.cumsum(counts[:-1], out=first[1:])
    pos = np.arange(E, dtype=np.int64) - first[b_s]

    core_s = core[order]
    win_s = win[order]
    chunk_s = chunk[order]
    cidx_s = cidx[order]
    dstl_s = dstl[order]
    w_s = invdeg[row[order]]

    # dw tables: [core, 128, WPC*T*2] fp16; defaults dst=-1 (never matches), w=0
    DWCOLS = WPC * T * 2
    dw = np.zeros((N_CORES, P, DWCOLS), dtype=np.float32)
    dw[:, :, 0::2] = -1.0
    tw = win_s * T + tile_base[chunk_s] + (pos >> 7)
    p_of = pos & 127
    dw[core_s, p_of, tw * 2] = dstl_s
    dw[core_s, p_of, tw * 2 + 1] = w_s

    # idx tables: per (core, group, chunk) call block of 16*t_c[c] cols,
    # index i at [i%16, col0 + i//16], replicated 8x across partitions.
    IDXCOLS = NGRP * 16 * T
    idx16 = np.zeros((N_CORES, 16, IDXCOLS), dtype=np.int16)
    g_s = win_s >> 1
    j_s = win_s & 1
    i_call = j_s * capE[chunk_s] + pos
    colg = g_s * 16 * T + 16 * tile_base[chunk_s] + (i_call >> 4)
    idx16[core_s, i_call & 15, colg] = cidx_s
    idx128 = np.tile(idx16, (1, 8, 1))

    x16 = np.ascontiguousarray(np.asarray(x, dtype=np.float16))
    W16 = np.ascontiguousarray(np.asarray(W, dtype=np.float16))
    b16 = np.ascontiguousarray(np.asarray(b, dtype=np.float16)).reshape(1, -1)

    per_core = []
    for cid in range(N_CORES):
        m = {
            "x": x16,
            "srcidx": np.ascontiguousarray(idx128[cid]),
            "dw": np.ascontiguousarray(dw[cid]),
            "Wm": W16,
            "bv": b16,
        }
        if have_mask:
            mw = (deg > 0).astype(np.float16).reshape(N_CORES, NPC)[cid]
            full = np.zeros((1, WPC * P), dtype=np.float16)
            full[0, :NPC] = mw
            m["maskw"] = full
        per_core.append(m)
    return per_core, t_c, have_mask


def run(x, edge_index, W, b, trace=False):
    in_maps, t_c, have_mask = prepare_inputs(x, edge_index, W, b)
    nc = build_nc(t_c, have_mask)
    nc.finalize()
    res = run_bass_kernel_spmd(nc, in_maps, list(range(N_CORES)), trace=trace)
    out = np.concatenate([res.results[c]["out"] for c in range(N_CORES)], axis=0)
    return out, res


def kernel(x, edge_index, W, b):
    out, _ = run(np.asarray(x), np.asarray(edge_index), np.asarray(W), np.asarray(b))
    return out.astype(np.float32)


# revision 15
# speedup vs baseline: 1.9881x; 1.6404x over previous
"""GraphSAGE mean-aggregation layer on 8 Trainium2 NeuronCores (raw Bass).

Math: out = D^{-1} A (x @ W + b)  ==  (D^{-1} A x) @ W + s (outer) b
where A is the (row=dest, col=src) adjacency from edge_index, D = row degrees,
s[d] = 1 if deg[d] > 0 else 0 (zero-degree rows are exactly 0 in the ref).

Strategy (one SPMD program on 8 cores, dest nodes sharded):
  - Host: sort edges by (dest window, src chunk). x is cast to fp16 and src
    rows are fetched with the gpsimd dma_gather custom instruction (int16
    indices => x is split into 4 chunks of 25000 rows). Each gather call
    covers one (window-pair, chunk) segment, capacity-padded with idx-0 /
    weight-0 edges so the SPMD program is identical on all cores.
  - Device, per 128-dest window: per 128-edge tile a DVE-built weighted
    one-hot S (S[e,j] = (dst_local[e]==j)*invdeg) in fp16, and PE matmuls
    G_half^T @ S accumulating the TRANSPOSED aggregate aggT [feat, dest]
    directly in PSUM (no separate transpose step). Scalar engine copies
    aggT to SBUF as fp16; PE then computes out = aggT^T @ W + s x b and the
    sync engine streams the result out.
"""

import numpy as np

import concourse.bass as bass
import concourse.mybir as mybir
from concourse import bacc
from concourse.bass_utils import run_bass_kernel_spmd

P = 128
F = 256

N_NODES = 100000
N_CORES = 8
NPC = N_NODES // N_CORES  # dest rows per core (12500)
WPC = (NPC + P - 1) // P  # dest windows per core (98)
ROWS_LAST = NPC - (WPC - 1) * P  # valid rows in last window (84)
NCH = 4
CH = N_NODES // NCH  # chunk rows (25000), int16-indexable
GW = 2  # windows per gather group
NGRP = WPC // GW  # gather groups per core (49)


def build_nc(t_c, have_mask):
    """One SPMD Bass program. t_c: tiles per chunk segment (len 4)."""
    t_c = list(t_c)
    T = sum(t_c)  # tiles per window
    tile_base = [0]
    for c in range(NCH):
        tile_base.append(tile_base[-1] + t_c[c])
    SLOTS_G = GW * T  # gather-buffer slots per group
    CPG = sum((2 * t_c[c] + 7) // 8 for c in range(NCH))  # gather calls per group
    NS = 3 * T  # S-tile ring slots
    IDXCOLS = NGRP * 16 * T
    DWCOLS = WPC * T * 2

    dt_f32 = mybir.dt.float32
    dt_f16 = mybir.dt.float16
    dt_i16 = mybir.dt.int16

    nc = bacc.Bacc(num_swdge_queues=4)

    x_h = nc.declare_dram_parameter("x", [N_NODES, F], dt_f16, isOutput=False)
    idx_h = nc.declare_dram_parameter("srcidx", [P, IDXCOLS], dt_i16, isOutput=False)
    dw_h = nc.declare_dram_parameter("dw", [P, DWCOLS], dt_f32, isOutput=False)
    w_h = nc.declare_dram_parameter("Wm", [F, F], dt_f16, isOutput=False)
    b_h = nc.declare_dram_parameter("bv", [1, F], dt_f16, isOutput=False)
    if have_mask:
        msk_h = nc.declare_dram_parameter("maskw", [1, WPC * P], dt_f16, isOutput=False)
    out_h = nc.declare_dram_parameter("out", [NPC, F], dt_f32, isOutput=True)

    n_loads = 6 if have_mask else 5
    META_TGT = 16 * n_loads

    from contextlib import ExitStack

    ctx = ExitStack()
    with ctx:
        sb = lambda name, shape, dt: ctx.enter_context(nc.sbuf_tensor(name, shape, dt))
        ps = lambda name, shape: ctx.enter_context(nc.psum_tensor(name, shape, dt_f32))
        sem = lambda name: ctx.enter_context(nc.semaphore(name))

        iota16 = sb("iota16", [P, P], dt_f16)
        ones16 = sb("ones16", [1, P], dt_f16)
        w0 = sb("w0", [P, F], dt_f16)
        w1 = sb("w1", [P, F], dt_f16)
        b_sb = sb("b_sb", [1, F], dt_f16)
        if have_mask:
            msk_sb = sb("msk_sb", [1, WPC * P], dt_f16)
        idx_sb = sb("idx_sb", [P, IDXCOLS], dt_i16)
        dw_sb = sb("dw_sb", [P, DWCOLS], dt_f32)
        g_buf = sb("g_buf", [P, 2 * SLOTS_G * F], dt_f16)
        s_buf = sb("s_buf", [P, NS * P], dt_f16)
        aggT_sb = sb("aggT_sb", [P, 2 * 2 * P], dt_f16)  # [pair][half]
        out_sb = sb("out_sb", [P, 2 * F], dt_f32)
        aggT_ps = [
            [ps("aggT_ps00", [P, P]), ps("aggT_ps01", [P, P])],
            [ps("aggT_ps10", [P, P]), ps("aggT_ps11", [P, P])],
        ]
        out_ps = [ps("out_ps0", [P, F]), ps("out_ps1", [P, F])]

        SEM_META = sem("sem_meta")
        SEM_CONST = sem("sem_const")
        SEM_ONES = sem("sem_ones")
        SEM_G = [sem(f"sem_g{q}") for q in range(4)]
        SEM_S = sem("sem_s")
        SEM_MM = sem("sem_mm")
        SEM_CP = sem("sem_cp")
        SEM_FIN = sem("sem_fin")
        SEM_OUT = sem("sem_out")
        SEM_OD = [sem("sem_od0"), sem("sem_od1")]

        w_sb = [w0, w1]

        # Gather queue assignment: even groups on SWDGE queues {0,1}, odd on
        # {2,3}. At most one group is in flight per queue, so PE's
        # sum-to-total waits on the per-queue completion semaphores are
        # deterministic despite out-of-order DMA completions.
        qassign = {}
        cumq = []
        run = [0, 0, 0, 0]
        for g_ in range(NGRP):
            k_ = 0
            for c_ in range(NCH):
                for off_ in range(0, 2 * t_c[c_], 8):
                    q_ = 2 * (g_ % 2) + (k_ % 2)
                    qassign[(g_, c_, off_)] = q_
                    run[q_] += 1
                    k_ += 1
            cumq.append(tuple(run))

        def gslot(g, c, j, k):
            """Gather-buffer slot of window-pair-member j, chunk c, tile k."""
            return (g % 2) * SLOTS_G + 2 * tile_base[c] + j * t_c[c] + k

        with nc.Block() as block:

            @block.sync
            def _(sync):
                sync.dma_start(w0[:, :], w_h[0:P, :]).then_inc(SEM_META, 16)
                sync.dma_start(w1[:, :], w_h[P : 2 * P, :]).then_inc(SEM_META, 16)
                sync.dma_start(b_sb[:, :], b_h[:, :]).then_inc(SEM_META, 16)
                sync.dma_start(idx_sb[:, :], idx_h[:, :]).then_inc(SEM_META, 16)
                sync.dma_start(dw_sb[:, :], dw_h[:, :]).then_inc(SEM_META, 16)
                if have_mask:
                    sync.dma_start(msk_sb[:, :], msk_h[:, :]).then_inc(SEM_META, 16)
                for w in range(WPC):
                    rows = P if w < WPC - 1 else ROWS_LAST
                    ob = (w % 2) * F
                    sync.wait_ge(SEM_OUT, w + 1)
                    sync.dma_start(
                        out_h[w * P : w * P + rows, :], out_sb[:rows, ob : ob + F]
                    ).then_inc(SEM_OD[w % 2], 16)

            @block.gpsimd
            def _(gpsimd):
                gpsimd.iota(
                    iota16[:, :],
                    pattern=[[1, P]],
                    base=0,
                    channel_multiplier=0,
                    allow_small_or_imprecise_dtypes=True,
                ).then_inc(SEM_CONST, 1)
                gpsimd.wait_ge(SEM_META, META_TGT)
                ni_regs = {}
                for c in range(NCH):
                    for off in range(0, 2 * t_c[c], 8):
                        nt = min(8, 2 * t_c[c] - off)
                        if nt * P not in ni_regs:
                            ni_regs[nt * P] = gpsimd.to_reg(nt * P)
                for g in range(NGRP):
                    if g >= 2:
                        # group buffer g%2 free once windows 2g-4, 2g-3 consumed
                        gpsimd.wait_ge(SEM_MM, (2 * g - 2) * 2 * T)
                    for c in range(NCH):
                        base = gslot(g, c, 0, 0)
                        col0 = g * 16 * T + 16 * tile_base[c]
                        # dma_gather is limited to 1024 indices per call
                        for off in range(0, 2 * t_c[c], 8):
                            nt = min(8, 2 * t_c[c] - off)
                            ni = nt * P
                            out_ap = g_buf[
                                :, (base + off) * F : (base + off + nt) * F
                            ].rearrange("p (s e) -> p s e", e=F)
                            gpsimd.dma_gather(
                                out_ap,
                                x_h[c * CH : (c + 1) * CH, :],
                                idx_sb[:, col0 + off * 8 : col0 + off * 8 + nt * 8],
                                ni,
                                ni_regs[ni],
                                F,
                                queue_num=qassign[(g, c, off)],
                            ).then_inc(SEM_G[qassign[(g, c, off)]], 16)

            @block.vector
            def _(vector):
                vector.memset(ones16[:, :], 1.0).then_inc(SEM_ONES, 1)
                vector.wait_ge(SEM_META, META_TGT)
                vector.wait_ge(SEM_CONST, 1)
                for w in range(WPC):
                    if w >= 3:
                        vector.wait_ge(SEM_MM, (w - 2) * 2 * T)
                    for t in range(T):
                        i = w * T + t
                        s0 = (i % NS) * P
                        col = (w * T + t) * 2
                        vector.tensor_scalar(
                            out=s_buf[:, s0 : s0 + P],
                            in0=iota16[:, :],
                            scalar1=dw_sb[:, col : col + 1],
                            scalar2=dw_sb[:, col + 1 : col + 2],
                            op0=mybir.AluOpType.is_equal,
                            op1=mybir.AluOpType.mult,
                        ).then_inc(SEM_S, 1)

            @block.scalar
            def _(scalar):
                for w in range(WPC):
                    pair = w % 2
                    scalar.wait_ge(SEM_MM, (w + 1) * 2 * T)
                    for h in range(2):
                        a0 = (pair * 2 + h) * P
                        scalar.copy(
                            aggT_sb[:, a0 : a0 + P], aggT_ps[pair][h][:, :]
                        ).then_inc(SEM_CP, 1)
                    scalar.wait_ge(SEM_FIN, w + 1)
                    if w >= 2:
                        scalar.wait_ge(SEM_OD[w % 2], 16 * (w // 2))
                    scalar.copy(
                        out_sb[:, pair * F : (pair + 1) * F], out_ps[pair][:, :]
                    ).then_inc(SEM_OUT, 1)

            @block.tensor
            def _(tensor):
                def tail(w):
                    pair = w % 2
                    tensor.wait_ge(SEM_CP, 2 * w + 2)
                    if w >= 2:
                        tensor.wait_ge(SEM_OUT, w - 1)
                    for h in range(2):
                        a0 = (pair * 2 + h) * P
                        tensor.matmul(
                            out_ps[pair][:, :],
                            aggT_sb[:, a0 : a0 + P],
                            w_sb[h][:, :],
                            start=(h == 0),
                            stop=False,
                        )
                    stat = (
                        msk_sb[0:1, w * P : (w + 1) * P]
                        if have_mask
                        else ones16[0:1, :]
                    )
                    tensor.matmul(
                        out_ps[pair][:, :],
                        stat,
                        b_sb[0:1, :],
                        start=False,
                        stop=True,
                    ).then_inc(SEM_FIN, 1)

                tensor.wait_ge(SEM_META, META_TGT)
                tensor.wait_ge(SEM_CONST, 1)
                tensor.wait_ge(SEM_ONES, 1)
                for w in range(WPC):
                    g = w // 2
                    j = w % 2
                    pair = w % 2
                    if w >= 2:
                        tensor.wait_ge(SEM_CP, 2 * w - 2)  # aggT_ps[pair] free
                    tensor.wait_ge(SEM_S, (w + 1) * T)
                    if j == 0:
                        for q in range(4):
                            if cumq[g][q] > (0 if g == 0 else cumq[g - 1][q]):
                                tensor.wait_ge(SEM_G[q], 16 * cumq[g][q])
                    for c in range(NCH):
                        for k in range(t_c[c]):
                            t = tile_base[c] + k
                            slot = gslot(g, c, j, k)
                            s0 = ((w * T + t) % NS) * P
                            st = t == 0
                            sp = t == T - 1
                            for h in range(2):
                                tensor.matmul(
                                    aggT_ps[pair][h][:, :],
                                    g_buf[:, slot * F + h * P : slot * F + (h + 1) * P],
                                    s_buf[:, s0 : s0 + P],
                                    start=st,
                                    stop=sp,
                                ).then_inc(SEM_MM, 1)
                    if w >= 1:
                        tail(w - 1)
                tail(WPC - 1)

    return nc


def prepare_inputs(x, edge_index, W, b):
    """Host-side: bucket edges by (core, window, chunk) into capacity-padded
    gather segments; build int16 gather-index tables and per-tile
    (dst_local, invdeg) scalar tables."""
    n = N_NODES
    row = np.asarray(edge_index[0], dtype=np.int64)  # dest
    col = np.asarray(edge_index[1], dtype=np.int64)  # src
    E = row.shape[0]

    deg = np.bincount(row, minlength=n)
    invdeg = np.where(deg > 0, 1.0 / np.maximum(deg, 1), 0.0).astype(np.float32)
    have_mask = bool((deg == 0).any())

    core = row // NPC
    local = row - core * NPC
    win = local >> 7
    dstl = (local & 127).astype(np.float32)
    chunk = col // CH
    cidx = (col - chunk * CH).astype(np.int16)

    bucket = (core * WPC + win) * NCH + chunk
    order = np.argsort(bucket, kind="stable")
    b_s = bucket[order]
    NB = N_CORES * WPC * NCH
    counts = np.bincount(b_s, minlength=NB)
    t_c = [
        int(np.ceil(counts.reshape(N_CORES, WPC, NCH)[:, :, c].max() / P))
        for c in range(NCH)
    ]
    T = sum(t_c)
    tile_base = np.concatenate([[0], np.cumsum(t_c)]).astype(np.int64)
    capE = np.array(t_c, dtype=np.int64) * P

    first = np.zeros(NB, dtype=np.int64)
    np.cumsum(counts[:-1], out=first[1:])
    pos = np.arange(E, dtype=np.int64) - first[b_s]

    core_s = core[order]
    win_s = win[order]
    chunk_s = chunk[order]
    cidx_s = cidx[order]
    dstl_s = dstl[order]
    w_s = invdeg[row[order]]

    # dw tables: [core, 128, WPC*T*2] fp16; defaults dst=-1 (never matches), w=0
    DWCOLS = WPC * T * 2
    dw = np.zeros((N_CORES, P, DWCOLS), dtype=np.float32)
    dw[:, :, 0::2] = -1.0
    tw = win_s * T + tile_base[chunk_s] + (pos >> 7)
    p_of = pos & 127
    dw[core_s, p_of, tw * 2] = dstl_s
    dw[core_s, p_of, tw * 2 + 1] = w_s

    # idx tables: per (core, group, chunk) call block of 16*t_c[c] cols,
    # index i at [i%16, col0 + i//16], replicated 8x across partitions.
    IDXCOLS = NGRP * 16 * T
    idx16 = np.zeros((N_CORES, 16, IDXCOLS), dtype=np.int16)
    g_s = win_s >> 1
    j_s = win_s & 1
    i_call = j_s * capE[chunk_s] + pos
    colg = g_s * 16 * T + 16 * tile_base[chunk_s] + (i_call >> 4)
    idx16[core_s, i_call & 15, colg] = cidx_s
    idx128 = np.tile(idx16, (1, 8, 1))

    x16 = np.ascontiguousarray(np.asarray(x, dtype=np.float16))
    W16 = np.ascontiguousarray(np.asarray(W, dtype=np.float16))
    b16 = np.ascontiguousarray(np.asarray(b, dtype=np.float16)).reshape(1, -1)

    per_core = []
    for cid in range(N_CORES):
        m = {
            "x": x16,
            "srcidx": np.ascontiguousarray(idx128[cid]),
            "dw": np.ascontiguousarray(dw[cid]),
            "Wm": W16,
            "bv": b16,
        }
        if have_mask:
            mw = (deg > 0).astype(np.float16).reshape(N_CORES, NPC)[cid]
            full = np.zeros((1, WPC * P), dtype=np.float16)
            full[0, :NPC] = mw
            m["maskw"] = full
        per_core.append(m)
    return per_core, t_c, have_mask


def run(x, edge_index, W, b, trace=False):
    in_maps, t_c, have_mask = prepare_inputs(x, edge_index, W, b)
    nc = build_nc(t_c, have_mask)
    nc.finalize()
    res = run_bass_kernel_spmd(nc, in_maps, list(range(N_CORES)), trace=trace)
    out = np.concatenate([res.results[c]["out"] for c in range(N_CORES)], axis=0)
    return out, res


def kernel(x, edge_index, W, b):
    out, _ = run(np.asarray(x), np.asarray(edge_index), np.asarray(W), np.asarray(b))
    return out.astype(np.float32)


# revision 17
# speedup vs baseline: 2.4114x; 1.2129x over previous
"""GraphSAGE mean-aggregation layer on 8 Trainium2 NeuronCores (raw Bass).

Math: out = D^{-1} A (x @ W + b)  ==  (D^{-1} A x) @ W + s (outer) b
where A is the (row=dest, col=src) adjacency from edge_index, D = row degrees,
s[d] = 1 if deg[d] > 0 else 0 (zero-degree rows are exactly 0 in the ref).

Strategy (one SPMD program on 8 cores, dest nodes sharded):
  - Host: sort edges by (dest window, src chunk). x is cast to fp16 and src
    rows are fetched with the gpsimd dma_gather custom instruction (int16
    indices => x is split into 4 chunks of 25000 rows). Each gather call
    covers one (window-pair, chunk) segment, capacity-padded with idx-0 /
    weight-0 edges so the SPMD program is identical on all cores.
  - Device, per 128-dest window: per 128-edge tile a DVE-built weighted
    one-hot S (S[e,j] = (dst_local[e]==j)*invdeg) in fp16, and PE matmuls
    G_half^T @ S accumulating the TRANSPOSED aggregate aggT [feat, dest]
    directly in PSUM (no separate transpose step). Scalar engine copies
    aggT to SBUF as fp16; PE then computes out = aggT^T @ W + s x b and the
    sync engine streams the result out.
"""

import numpy as np

import concourse.bass as bass
import concourse.mybir as mybir
from concourse import bacc
from concourse.bass_utils import run_bass_kernel_spmd

P = 128
F = 256

N_NODES = 100000
N_CORES = 8
NPC = N_NODES // N_CORES  # dest rows per core (12500)
WPC = (NPC + P - 1) // P  # dest windows per core (98)
ROWS_LAST = NPC - (WPC - 1) * P  # valid rows in last window (84)
NCH = 4
CH = N_NODES // NCH  # chunk rows (25000), int16-indexable
GW = 2  # windows per gather group
NGRP = WPC // GW  # gather groups per core (49)


def build_nc(t_c, have_mask):
    """One SPMD Bass program. t_c: tiles per chunk segment (len 4)."""
    t_c = list(t_c)
    T = sum(t_c)  # tiles per window
    tile_base = [0]
    for c in range(NCH):
        tile_base.append(tile_base[-1] + t_c[c])
    SLOTS_G = GW * T  # gather-buffer slots per group
    CPG = sum((2 * t_c[c] + 7) // 8 for c in range(NCH))  # gather calls per group
    ACT_TILES = [t for t in range(T) if t % 3 == 2]  # S tiles built on Act
    NACT = len(ACT_TILES)
    NDVE = T - NACT
    ACTCOLS = WPC * NACT * 3
    NS = 3 * T  # S-tile ring slots
    IDXCOLS = NGRP * 16 * T
    DWCOLS = WPC * T * 2

    dt_f32 = mybir.dt.float32
    dt_f16 = mybir.dt.float16
    dt_i16 = mybir.dt.int16

    nc = bacc.Bacc(num_swdge_queues=4)

    x_h = nc.declare_dram_parameter("x", [N_NODES, F], dt_f16, isOutput=False)
    idx_h = nc.declare_dram_parameter("srcidx", [P, IDXCOLS], dt_i16, isOutput=False)
    dw_h = nc.declare_dram_parameter("dw", [P, DWCOLS], dt_f32, isOutput=False)
    adw_h = nc.declare_dram_parameter("adw", [P, ACTCOLS], dt_f32, isOutput=False)
    w_h = nc.declare_dram_parameter("Wm", [F, F], dt_f16, isOutput=False)
    b_h = nc.declare_dram_parameter("bv", [1, F], dt_f16, isOutput=False)
    if have_mask:
        msk_h = nc.declare_dram_parameter("maskw", [1, WPC * P], dt_f16, isOutput=False)
    out_h = nc.declare_dram_parameter("out", [NPC, F], dt_f32, isOutput=True)

    n_loads = 7 if have_mask else 6
    META_TGT = 16 * n_loads

    from contextlib import ExitStack

    ctx = ExitStack()
    with ctx:
        sb = lambda name, shape, dt: ctx.enter_context(nc.sbuf_tensor(name, shape, dt))
        ps = lambda name, shape: ctx.enter_context(nc.psum_tensor(name, shape, dt_f32))
        sem = lambda name: ctx.enter_context(nc.semaphore(name))

        iota16 = sb("iota16", [P, P], dt_f16)
        ones16 = sb("ones16", [1, P], dt_f16)
        w0 = sb("w0", [P, F], dt_f16)
        w1 = sb("w1", [P, F], dt_f16)
        b_sb = sb("b_sb", [1, F], dt_f16)
        if have_mask:
            msk_sb = sb("msk_sb", [1, WPC * P], dt_f16)
        idx_sb = sb("idx_sb", [P, IDXCOLS], dt_i16)
        dw_sb = sb("dw_sb", [P, DWCOLS], dt_f32)
        adw_sb = sb("adw_sb", [P, ACTCOLS], dt_f32)
        u_sb = sb("u_sb", [P, 2 * P], dt_f16)
        g_buf = sb("g_buf", [P, 2 * SLOTS_G * F], dt_f16)
        s_buf = sb("s_buf", [P, NS * P], dt_f16)
        aggT_sb = sb("aggT_sb", [P, 2 * 2 * P], dt_f16)  # [pair][half]
        out_sb = sb("out_sb", [P, 2 * F], dt_f32)
        aggT_ps = [
            [ps("aggT_ps00", [P, P]), ps("aggT_ps01", [P, P])],
            [ps("aggT_ps10", [P, P]), ps("aggT_ps11", [P, P])],
        ]
        out_ps = [ps("out_ps0", [P, F]), ps("out_ps1", [P, F])]

        SEM_META = sem("sem_meta")
        SEM_CONST = sem("sem_const")
        SEM_ONES = sem("sem_ones")
        SEM_G = [sem(f"sem_g{q}") for q in range(4)]
        SEM_S = sem("sem_s")
        SEM_SA = sem("sem_sa")
        SEM_U = sem("sem_u")
        SEM_MM = sem("sem_mm")
        SEM_CP = sem("sem_cp")
        SEM_FIN = sem("sem_fin")
        SEM_OUT = sem("sem_out")
        SEM_OD = [sem("sem_od0"), sem("sem_od1")]

        w_sb = [w0, w1]

        # Gather queue assignment: even groups on SWDGE queues {0,1}, odd on
        # {2,3}. At most one group is in flight per queue, so PE's
        # sum-to-total waits on the per-queue completion semaphores are
        # deterministic despite out-of-order DMA completions.
        qassign = {}
        cumq = []
        run = [0, 0, 0, 0]
        for g_ in range(NGRP):
            k_ = 0
            for c_ in range(NCH):
                for off_ in range(0, 2 * t_c[c_], 8):
                    q_ = 2 * (g_ % 2) + (k_ % 2)
                    qassign[(g_, c_, off_)] = q_
                    run[q_] += 1
                    k_ += 1
            cumq.append(tuple(run))

        def gslot(g, c, j, k):
            """Gather-buffer slot of window-pair-member j, chunk c, tile k."""
            return (g % 2) * SLOTS_G + 2 * tile_base[c] + j * t_c[c] + k

        with nc.Block() as block:

            @block.sync
            def _(sync):
                sync.dma_start(w0[:, :], w_h[0:P, :]).then_inc(SEM_META, 16)
                sync.dma_start(w1[:, :], w_h[P : 2 * P, :]).then_inc(SEM_META, 16)
                sync.dma_start(b_sb[:, :], b_h[:, :]).then_inc(SEM_META, 16)
                sync.dma_start(idx_sb[:, :], idx_h[:, :]).then_inc(SEM_META, 16)
                sync.dma_start(dw_sb[:, :], dw_h[:, :]).then_inc(SEM_META, 16)
                sync.dma_start(adw_sb[:, :], adw_h[:, :]).then_inc(SEM_META, 16)
                if have_mask:
                    sync.dma_start(msk_sb[:, :], msk_h[:, :]).then_inc(SEM_META, 16)
                for w in range(WPC):
                    rows = P if w < WPC - 1 else ROWS_LAST
                    ob = (w % 2) * F
                    sync.wait_ge(SEM_OUT, w + 1)
                    sync.dma_start(
                        out_h[w * P : w * P + rows, :], out_sb[:rows, ob : ob + F]
                    ).then_inc(SEM_OD[w % 2], 16)

            @block.gpsimd
            def _(gpsimd):
                gpsimd.iota(
                    iota16[:, :],
                    pattern=[[1, P]],
                    base=0,
                    channel_multiplier=0,
                    allow_small_or_imprecise_dtypes=True,
                ).then_inc(SEM_CONST, 1)
                gpsimd.wait_ge(SEM_META, META_TGT)
                ni_regs = {}
                for c in range(NCH):
                    for off in range(0, 2 * t_c[c], 8):
                        nt = min(8, 2 * t_c[c] - off)
                        if nt * P not in ni_regs:
                            ni_regs[nt * P] = gpsimd.to_reg(nt * P)
                for g in range(NGRP):
                    if g >= 2:
                        # group buffer g%2 free once windows 2g-4, 2g-3 consumed
                        gpsimd.wait_ge(SEM_MM, (2 * g - 2) * 2 * T)
                    for c in range(NCH):
                        base = gslot(g, c, 0, 0)
                        col0 = g * 16 * T + 16 * tile_base[c]
                        # dma_gather is limited to 1024 indices per call
                        for off in range(0, 2 * t_c[c], 8):
                            nt = min(8, 2 * t_c[c] - off)
                            ni = nt * P
                            out_ap = g_buf[
                                :, (base + off) * F : (base + off + nt) * F
                            ].rearrange("p (s e) -> p s e", e=F)
                            gpsimd.dma_gather(
                                out_ap,
                                x_h[c * CH : (c + 1) * CH, :],
                                idx_sb[:, col0 + off * 8 : col0 + off * 8 + nt * 8],
                                ni,
                                ni_regs[ni],
                                F,
                                queue_num=qassign[(g, c, off)],
                            ).then_inc(SEM_G[qassign[(g, c, off)]], 16)

            @block.vector
            def _(vector):
                vector.memset(ones16[:, :], 1.0).then_inc(SEM_ONES, 1)
                vector.wait_ge(SEM_META, META_TGT)
                vector.wait_ge(SEM_CONST, 1)
                for w in range(WPC):
                    if w >= 3:
                        vector.wait_ge(SEM_MM, (w - 2) * 2 * T)
                    for t in range(T):
                        if t % 3 == 2:
                            continue
                        i = w * T + t
                        s0 = (i % NS) * P
                        col = (w * T + t) * 2
                        vector.tensor_scalar(
                            out=s_buf[:, s0 : s0 + P],
                            in0=iota16[:, :],
                            scalar1=dw_sb[:, col : col + 1],
                            scalar2=dw_sb[:, col + 1 : col + 2],
                            op0=mybir.AluOpType.is_equal,
                            op1=mybir.AluOpType.mult,
                        ).then_inc(SEM_S, 1)

            @block.scalar
            def _(scalar):
                actr = [0]

                def build_s(w):
                    # S = Relu((-w) * |iota - dst| + w), via 2 activations.
                    # u ping-pongs between 2 buffers; SEM_U/SEM_SA order the
                    # same-engine RAW/WAR hazards on u.
                    for k, t in enumerate(ACT_TILES):
                        n = actr[0]
                        actr[0] += 1
                        i = w * T + t
                        s0 = (i % NS) * P
                        u0 = (n % 2) * P
                        col = (w * NACT + k) * 3
                        if n >= 2:
                            scalar.wait_ge(SEM_SA, n - 1)
                        scalar.activation(
                            u_sb[:, u0 : u0 + P],
                            iota16[:, :],
                            mybir.ActivationFunctionType.Abs,
                            bias=adw_sb[:, col : col + 1],
                        ).then_inc(SEM_U, 1)
                        scalar.wait_ge(SEM_U, n + 1)
                        scalar.activation(
                            s_buf[:, s0 : s0 + P],
                            u_sb[:, u0 : u0 + P],
                            mybir.ActivationFunctionType.Relu,
                            bias=adw_sb[:, col + 2 : col + 3],
                            scale=adw_sb[:, col + 1 : col + 2],
                        ).then_inc(SEM_SA, 1)

                scalar.wait_ge(SEM_META, META_TGT)
                scalar.wait_ge(SEM_CONST, 1)
                build_s(0)
                build_s(1)
                for w in range(WPC):
                    pair = w % 2
                    scalar.wait_ge(SEM_MM, (w + 1) * 2 * T)
                    for h in range(2):
                        a0 = (pair * 2 + h) * P
                        scalar.copy(
                            aggT_sb[:, a0 : a0 + P], aggT_ps[pair][h][:, :]
                        ).then_inc(SEM_CP, 1)
                    if w + 2 < WPC:
                        build_s(w + 2)
                    scalar.wait_ge(SEM_FIN, w + 1)
                    if w >= 2:
                        scalar.wait_ge(SEM_OD[w % 2], 16 * (w // 2))
                    scalar.copy(
                        out_sb[:, pair * F : (pair + 1) * F], out_ps[pair][:, :]
                    ).then_inc(SEM_OUT, 1)

            @block.tensor
            def _(tensor):
                def tail(w):
                    pair = w % 2
                    tensor.wait_ge(SEM_CP, 2 * w + 2)
                    if w >= 2:
                        tensor.wait_ge(SEM_OUT, w - 1)
                    for h in range(2):
                        a0 = (pair * 2 + h) * P
                        tensor.matmul(
                            out_ps[pair][:, :],
                            aggT_sb[:, a0 : a0 + P],
                            w_sb[h][:, :],
                            start=(h == 0),
                            stop=False,
                        )
                    stat = (
                        msk_sb[0:1, w * P : (w + 1) * P]
                        if have_mask
                        else ones16[0:1, :]
                    )
                    tensor.matmul(
                        out_ps[pair][:, :],
                        stat,
                        b_sb[0:1, :],
                        start=False,
                        stop=True,
                    ).then_inc(SEM_FIN, 1)

                tensor.wait_ge(SEM_META, META_TGT)
                tensor.wait_ge(SEM_CONST, 1)
                tensor.wait_ge(SEM_ONES, 1)
                for w in range(WPC):
                    g = w // 2
                    j = w % 2
                    pair = w % 2
                    if w >= 2:
                        tensor.wait_ge(SEM_CP, 2 * w - 2)  # aggT_ps[pair] free
                    tensor.wait_ge(SEM_S, (w + 1) * NDVE)
                    tensor.wait_ge(SEM_SA, (w + 1) * NACT)
                    if j == 0:
                        for q in range(4):
                            if cumq[g][q] > (0 if g == 0 else cumq[g - 1][q]):
                                tensor.wait_ge(SEM_G[q], 16 * cumq[g][q])
                    for c in range(NCH):
                        for k in range(t_c[c]):
                            t = tile_base[c] + k
                            slot = gslot(g, c, j, k)
                            s0 = ((w * T + t) % NS) * P
                            st = t == 0
                            sp = t == T - 1
                            for h in range(2):
                                tensor.matmul(
                                    aggT_ps[pair][h][:, :],
                                    g_buf[:, slot * F + h * P : slot * F + (h + 1) * P],
                                    s_buf[:, s0 : s0 + P],
                                    start=st,
                                    stop=sp,
                                ).then_inc(SEM_MM, 1)
                    if w >= 1:
                        tail(w - 1)
                tail(WPC - 1)

    return nc


def prepare_inputs(x, edge_index, W, b):
    """Host-side: bucket edges by (core, window, chunk) into capacity-padded
    gather segments; build int16 gather-index tables and per-tile
    (dst_local, invdeg) scalar tables."""
    n = N_NODES
    row = np.asarray(edge_index[0], dtype=np.int64)  # dest
    col = np.asarray(edge_index[1], dtype=np.int64)  # src
    E = row.shape[0]

    deg = np.bincount(row, minlength=n)
    invdeg = np.where(deg > 0, 1.0 / np.maximum(deg, 1), 0.0).astype(np.float32)
    have_mask = bool((deg == 0).any())

    core = row // NPC
    local = row - core * NPC
    win = local >> 7
    dstl = (local & 127).astype(np.float32)
    chunk = col // CH
    cidx = (col - chunk * CH).astype(np.int16)

    bucket = (core * WPC + win) * NCH + chunk
    order = np.argsort(bucket, kind="stable")
    b_s = bucket[order]
    NB = N_CORES * WPC * NCH
    counts = np.bincount(b_s, minlength=NB)
    t_c = [
        int(np.ceil(counts.reshape(N_CORES, WPC, NCH)[:, :, c].max() / P))
        for c in range(NCH)
    ]
    T = sum(t_c)
    tile_base = np.concatenate([[0], np.cumsum(t_c)]).astype(np.int64)
    capE = np.array(t_c, dtype=np.int64) * P

    first = np.zeros(NB, dtype=np.int64)
    np.cumsum(counts[:-1], out=first[1:])
    pos = np.arange(E, dtype=np.int64) - first[b_s]

    core_s = core[order]
    win_s = win[order]
    chunk_s = chunk[order]
    cidx_s = cidx[order]
    dstl_s = dstl[order]
    w_s = invdeg[row[order]]

    # dw tables: [core, 128, WPC*T*2] fp32; defaults dst=-1 (never matches), w=0.
    # Every 3rd tile (t%3==2) is built on the Act engine from adw instead:
    # (-dst, -w, w) for S = Relu(-w*|iota-dst| + w).
    DWCOLS = WPC * T * 2
    NACT = len([t for t in range(T) if t % 3 == 2])
    ACTCOLS = WPC * NACT * 3
    dw = np.zeros((N_CORES, P, DWCOLS), dtype=np.float32)
    dw[:, :, 0::2] = -1.0
    adw = np.zeros((N_CORES, P, ACTCOLS), dtype=np.float32)
    adw[:, :, 0::3] = 1.0  # ndst=-dst default dst=-1
    tile = tile_base[chunk_s] + (pos >> 7)
    tw = win_s * T + tile
    p_of = pos & 127
    is_act = (tile % 3) == 2
    dv = ~is_act
    dw[core_s[dv], p_of[dv], tw[dv] * 2] = dstl_s[dv]
    dw[core_s[dv], p_of[dv], tw[dv] * 2 + 1] = w_s[dv]
    ak = (win_s * NACT + tile // 3) * 3
    dw_a = core_s[is_act], p_of[is_act]
    adw[dw_a[0], dw_a[1], ak[is_act]] = -dstl_s[is_act]
    adw[dw_a[0], dw_a[1], ak[is_act] + 1] = -w_s[is_act]
    adw[dw_a[0], dw_a[1], ak[is_act] + 2] = w_s[is_act]

    # idx tables: per (core, group, chunk) call block of 16*t_c[c] cols,
    # index i at [i%16, col0 + i//16], replicated 8x across partitions.
    IDXCOLS = NGRP * 16 * T
    idx16 = np.zeros((N_CORES, 16, IDXCOLS), dtype=np.int16)
    g_s = win_s >> 1
    j_s = win_s & 1
    i_call = j_s * capE[chunk_s] + pos
    colg = g_s * 16 * T + 16 * tile_base[chunk_s] + (i_call >> 4)
    idx16[core_s, i_call & 15, colg] = cidx_s
    idx128 = np.tile(idx16, (1, 8, 1))

    x16 = np.ascontiguousarray(np.asarray(x, dtype=np.float16))
    W16 = np.ascontiguousarray(np.asarray(W, dtype=np.float16))
    b16 = np.ascontiguousarray(np.asarray(b, dtype=np.float16)).reshape(1, -1)

    per_core = []
    for cid in range(N_CORES):
        m = {
            "x": x16,
            "srcidx": np.ascontiguousarray(idx128[cid]),
            "dw": np.ascontiguousarray(dw[cid]),
            "adw": np.ascontiguousarray(adw[cid]),
            "Wm": W16,
            "bv": b16,
        }
        if have_mask:
            mw = (deg > 0).astype(np.float16).reshape(N_CORES, NPC)[cid]
            full = np.zeros((1, WPC * P), dtype=np.float16)
            full[0, :NPC] = mw
            m["maskw"] = full
        per_core.append(m)
    return per_core, t_c, have_mask


def run(x, edge_index, W, b, trace=False):
    in_maps, t_c, have_mask = prepare_inputs(x, edge_index, W, b)
    nc = build_nc(t_c, have_mask)
    nc.finalize()
    res = run_bass_kernel_spmd(nc, in_maps, list(range(N_CORES)), trace=trace)
    out = np.concatenate([res.results[c]["out"] for c in range(N_CORES)], axis=0)
    return out, res


def kernel(x, edge_index, W, b):
    out, _ = run(np.asarray(x), np.asarray(edge_index), np.asarray(W), np.asarray(b))
    return out.astype(np.float32)
